# revision 16
# baseline (speedup 1.0000x reference)
"""Trainium2 Bass kernel: per-batch cosine-distance matrix.

out[b] = 1 - metric[b] @ metric[b].T   where metric = x / ||x||_2 (last dim)
x: [32, 1024, 768] f32  ->  out: [32, 1024, 1024] f32

Sharding: data-parallel over batch. 8 cores x 4 batches each; no
cross-core communication.

Design (raw-fp8 gram, SBUF-staged scale-at-eviction; v8):
  Host prep: cast x to fp8 e4m3, transpose each batch to xT8 [C, T]
  (layout/dtype prep only - all math runs on device). Per batch,
  row blocks m = 0..7 (upper triangle incl. diagonal, cols m*128..T):
    1. DMA xT8 -> SBUF x83 [128, (k, t)] fp8, 6 chunked DMAs over the
       SP+ACT hwdge queues.
    2. PE: gram row m on RAW x8 (no met8 prescale): psum chunks of
       <=512 f32. fp8 DoubleRow K=256/matmul, j-outer so the chunks of
       a (m, j) pair share one LDWEIGHTS: the tile legalizer emits one
       LDWEIGHTS per matmul, so dedup_ldweights() strips the redundant
       loads post-legalize (migrating waits/deps); every PE instruction
       is chained (chain_iter_dep) so the final PE order is exactly
       emission order, making the weight reuse sound.
    3. Each psum chunk is immediately staged to SBUF bf16 (DVE
       tensor_copy / ACT copy split) into raw[b] [128, 4608] - psum
       slots recycle within ~1us, so the PE never waits on the
       normalization chain. 1-bank chunk slots, bufs=6.
    4. Norms: DVE stt-vs-identity on the psum diagonal block
       -> rv[:, m] = ss. Once per batch: DVE reciprocal (rr = 1/ss),
       then PE transpose of rr [128,8]->[8,128] - its only dependency
       is the DVE reciprocal, so the chained PE never waits on the busy
       ACT queue; the sqrt rides the row8 copy AFTER the transpose:
       row8 = Sqrt(rr^T) = +rinv bf16 -> DRAM -> partition-broadcast
       -> RI [128, T] bf16. nrivs = -rinv f32 is the per-partition
       eviction scale (two small ACT ops, off the critical path).
    5. Evictions of batch b run during batch b+1, emitted 3+ iterations
       in (avoids DVE FIFO head-of-line on RI): DVE tensor_tensor
       sc = raw * RI (all-bf16 SBUF), then ob = sc * nrivs[:, m] + 1.0
       -> f16 on ACT (rows 0-1, activation with AP scale) or Pool
       (rows 2-7, tensor_scalar) to balance engines; DMA out on
       alternating queues. The final batch's evictions (the tail) are
       split ACT/DVE/Pool three ways.
  Host post: upcast f16 -> f32 and mirror the (symmetric) lower half.

Measured on 8 axon trn2 cores: 76.8-77.6 us HW exec (met8 baseline:
87.3-89.8 us), rel err 1.38e-3 (budget 2e-2). Engine busy at 77 us:
DVE 60% / PE 47% (+20% LDWEIGHTS) / ACT 35% / Pool 20%.
"""

import sys
import time
from contextlib import ExitStack

_TRN_REPO = "/opt/trn_rl_repo"
if _TRN_REPO not in sys.path:
    sys.path.insert(0, _TRN_REPO)

import numpy as np
import ml_dtypes

import concourse.bacc as bacc
import concourse.mybir as mybir
import concourse.tile as tile
from concourse.bass_utils import run_bass_kernel_spmd
from concourse.masks import make_identity

B, T, C = 32, 1024, 768
N_CORES = 8
BPC = B // N_CORES   # batches per core
KC = C // 128        # 6 k-chunks
KP = KC // 2         # 3 k-pairs (DoubleRow)
TT = T // 128        # 8 row blocks
F32 = mybir.dt.float32
F16 = mybir.dt.float16
BF16 = mybir.dt.bfloat16
F8 = mybir.dt.float8e4
AF = mybir.ActivationFunctionType
ALU = mybir.AluOpType
DR = mybir.MatmulPerfMode.DoubleRow

ROW_OFF = [0]
for _m in range(TT):
    ROW_OFF.append(ROW_OFF[-1] + (T - _m * 128))  # prefix offsets into raw


def dedup_ldweights(nc):
    """Remove InstLdweights whose weights AP equals the currently-loaded
    stationary (set by the previous LDW and not clobbered since). Runs
    after tile legalization (final instruction order) and before
    nc.compile() (semaphore generation), so migrating the removed LDW's
    sync_info and dependency edges onto the following matmul is safe.
    """
    removed = 0
    for f in nc.m.functions:
        for bb in f.blocks:
            keep = []
            last_key = None
            donors = []
            for inst in bb.instructions:
                if getattr(inst, "engine", None) != mybir.EngineType.PE:
                    keep.append(inst)
                    continue
                if isinstance(inst, mybir.InstLdweights):
                    try:
                        key = (str(inst.ins[0]), str(inst.perf_mode))
                    except Exception:
                        key = None
                    if key is not None and key == last_key:
                        donors.append(inst)
                        removed += 1
                        continue
                    last_key = key
                    keep.append(inst)
                else:
                    if isinstance(inst, mybir.InstMatmult) and inst.is_transpose:
                        last_key = None
                    elif not isinstance(inst, mybir.InstMatmult):
                        last_key = None
                    for d in donors:
                        inst.merge_dependencies_from(d)
                        dsi = d.sync_info
                        if dsi is not None and (dsi.on_wait or dsi.on_update):
                            si = inst.sync_info
                            ow = list(si.on_wait) if si else []
                            ou = list(si.on_update) if si else []
                            inst.sync_info = mybir.SyncInfo(
                                on_wait=ow + list(dsi.on_wait),
                                on_update=ou + list(dsi.on_update))
                    donors = []
                    keep.append(inst)
            assert not donors, "trailing removed LDW with no successor"
            bb.instructions[:] = keep
    return removed


def build():
    nc = bacc.Bacc("TRN2", target_bir_lowering=False, debug=False,
                   num_devices=N_CORES)
    xT8 = nc.dram_tensor("xT8", [BPC, C, T], F8, kind="ExternalInput").ap()
    out = nc.dram_tensor("out", [BPC, T, T], F16, kind="ExternalOutput").ap()
    rowsc = nc.dram_tensor("rowsc", [BPC, T], BF16, kind="Internal").ap()

    with tile.TileContext(nc) as tc, ExitStack() as ctx:
        x_pool = ctx.enter_context(tc.tile_pool(name="x", bufs=3))
        raw_pool = ctx.enter_context(tc.tile_pool(name="raw", bufs=2))
        ri_pool = ctx.enter_context(tc.tile_pool(name="ri", bufs=2))
        s_pool = ctx.enter_context(tc.tile_pool(name="s", bufs=2))
        sc_pool = ctx.enter_context(tc.tile_pool(name="sc", bufs=3))
        ob_pool = ctx.enter_context(tc.tile_pool(name="ob", bufs=4))
        c_pool = ctx.enter_context(tc.tile_pool(name="c", bufs=1))
        ps_pool = ctx.enter_context(
            tc.tile_pool(name="ps", bufs=6, space="PSUM"))  # 1 bank/slot
        psT_pool = ctx.enter_context(
            tc.tile_pool(name="psT", bufs=1, space="PSUM"))

        identf = c_pool.tile([128, 128], F32)
        make_identity(nc, identf[:])
        dummyf = c_pool.tile([128, 128], F32, tag="dummyf")

        # warm the ACT Sqrt table while the first DMA flies
        warm = c_pool.tile([128, 1], F32, tag="warm")
        nc.vector.memset(warm[:], 1.0)
        warm2 = c_pool.tile([128, 1], F32, tag="warm2")
        nc.scalar.sqrt(warm2[:], warm[:])

        x83s, raws, RIs, rvs, rrs, rivs, nrivs, rvTs = ({}, {}, {}, {}, {}, {}, {}, {})

        def chain(inst):
            tc.chain_iter_dep("pe_order", inst.ins)

        def emit_load(b):
            x8 = x_pool.tile([128, KC * T], F8, tag="x8", name=f"x8_{b}")
            x83 = x8[:].rearrange("p (k t) -> p k t", k=KC)
            src = xT8[b].rearrange("(k p) t -> p k t", p=128)
            for k in range(KC):
                eng = nc.sync if k % 2 == 0 else nc.scalar
                eng.dma_start(x83[:, k, :], src[:, k, :])
            x83s[b] = x83

        def alloc_batch(b):
            raws[b] = raw_pool.tile([128, ROW_OFF[TT]], BF16, tag="raw",
                                    name=f"raw_{b}")
            RIs[b] = ri_pool.tile([128, T], BF16, tag="RI", name=f"RI_{b}")
            rvs[b] = s_pool.tile([128, TT], F32, tag="rv", name=f"rv_{b}")
            rrs[b] = s_pool.tile([128, TT], F32, tag="rr", name=f"rr_{b}")
            rivs[b] = s_pool.tile([128, TT], F32, tag="riv", name=f"riv_{b}")
            nrivs[b] = s_pool.tile([128, TT], F32, tag="nriv",
                                   name=f"nriv_{b}")

        def chunks_of(m):
            W = T - m * 128
            ch = [(0, min(512, W))]
            if W > 512:
                ch.append((512, W - 512))
            return ch

        def emit_diag_pass(b):
            # last batch only: compute all 8 diagonal blocks up front so
            # the norm chain (and hence this batch's evictions) overlaps
            # this batch's own gram pass instead of trailing the kernel.
            x83 = x83s[b]
            dts = [ps_pool.tile([128, 512], F32, tag="pg",
                                name=f"dg_{b}_{h}") for h in range(2)]
            for m in range(TT):
                dst = dts[m // 4][:, (m % 4) * 128:(m % 4) * 128 + 128]
                msl = slice(m * 128, (m + 1) * 128)
                for j in range(KP):
                    mm = nc.tensor.matmul(
                        dst, x83[:, 2 * j:2 * j + 2, msl],
                        x83[:, 2 * j:2 * j + 2, msl],
                        start=(j == 0), stop=(j == KP - 1),
                        perf_mode=DR, skip_group_check=True)
                    chain(mm)
                nc.vector.scalar_tensor_tensor(
                    dummyf[:], dst, 1.0, identf[:], ALU.mult,
                    ALU.mult, accum_out=rvs[b][:, m:m + 1])

        def pe_group(b, m, extract=True):
            # gram row m on raw x8, j-outer; chunk tiles are 1 psum bank;
            # each chunk staged to SBUF bf16 right after its stop matmul.
            x83 = x83s[b]
            n0 = m * 128
            tiles = []
            for ci, (off, w) in enumerate(chunks_of(m)):
                pgt = ps_pool.tile([128, w], F32, tag="pg",
                                   name=f"pg_{b}_{m}_{ci}",
                                   padded_shape=[128, 512])
                tiles.append((pgt, off, w))
            pgs0 = tiles[0][0]
            msl = slice(n0, n0 + 128)
            for j in range(KP):
                for pgt, off, w in tiles:
                    mm = nc.tensor.matmul(
                        pgt[:],
                        x83[:, 2 * j:2 * j + 2, msl],
                        x83[:, 2 * j:2 * j + 2, n0 + off:n0 + off + w],
                        start=(j == 0), stop=(j == KP - 1),
                        perf_mode=DR, skip_group_check=True)
                    chain(mm)
            if extract:
                # ss from the psum diagonal block, before staging
                nc.vector.scalar_tensor_tensor(
                    dummyf[:], pgs0[:, 0:128], 1.0, identf[:], ALU.mult,
                    ALU.mult, accum_out=rvs[b][:, m:m + 1])
            # stage to SBUF: DVE takes the first (diagonal-bearing) chunk,
            # ACT the rest, so DVE keeps psum slots rolling without the
            # staging backlog queuing behind eviction tensor_tensors.
            for pgt, off, w in tiles:
                dst = raws[b][:, ROW_OFF[m] + off:ROW_OFF[m] + off + w]
                if m >= 4 or off > 0:
                    nc.scalar.activation(dst, pgt[:], AF.Copy, bias=0.0,
                                         scale=1.0)
                else:
                    nc.vector.tensor_copy(dst, pgt[:])

        def emit_norm_head(b):
            # once per batch, right after the last extract. Only the DVE
            # reciprocal sits ahead of the PE transpose; sqrt happens
            # after the transpose so the chained PE never waits on ACT.
            nc.vector.reciprocal(rrs[b][:], rvs[b][:])

        def emit_norm_tail(b):
            # PE transpose (of rr = 1/ss) is emitted by the caller; here:
            # row8 = sqrt(rr)^T = +rinv (bf16), DRAM round trip, partition
            # broadcast -> RI (positive). nrivs = -rinv is the per-partition
            # eviction scale; neither is on the PE critical path.
            row8 = s_pool.tile([TT, 128], BF16, tag="row8",
                               name=f"row8_{b}")
            nc.scalar.activation(row8[:], rvTs[b][:], AF.Sqrt, bias=0.0,
                                 scale=1.0)
            nc.sync.dma_start(rowsc[b], row8[:])
            nc.scalar.dma_start(
                RIs[b][:], rowsc[b].unsqueeze(0).to_broadcast((128, T)))
            nc.scalar.activation(rivs[b][:], rrs[b][:], AF.Sqrt, bias=0.0,
                                 scale=1.0)
            nc.scalar.activation(nrivs[b][:], rivs[b][:], AF.Copy, bias=0.0,
                                 scale=-1.0)

        def emit_transpose(b):
            rvT = psT_pool.tile([TT, 128], F32, tag="rvT", name=f"rvT_{b}")
            mmT = nc.tensor.transpose(rvT[:], rrs[b][:], identf[:])
            chain(mmT)
            rvTs[b] = rvT

        def emit_col_chain(b, m):
            # per-block norm chain (last batch): rr/rinv/-rinv columns
            nc.vector.reciprocal(rrs[b][:, m:m + 1], rvs[b][:, m:m + 1])
            nc.scalar.activation(rivs[b][:, m:m + 1], rrs[b][:, m:m + 1],
                                 AF.Sqrt, bias=0.0, scale=1.0)
            nc.scalar.activation(nrivs[b][:, m:m + 1], rivs[b][:, m:m + 1],
                                 AF.Copy, bias=0.0, scale=-1.0)

        def emit_col_transpose(b, m):
            # PE transpose of one rr column (chained, one group of slack),
            # sqrt rides the [1,128] copy, then the block's RI broadcast.
            rvT1 = psT_pool.tile([1, 128], F32, tag="rvT1",
                                 name=f"rvT1_{b}_{m}")
            mmT = nc.tensor.transpose(rvT1[:], rrs[b][:, m:m + 1], identf[:])
            chain(mmT)
            row1 = s_pool.tile([1, 128], BF16, tag="row1", bufs=3,
                               name=f"row1_{b}_{m}")
            nc.scalar.activation(row1[:], rvT1[:], AF.Sqrt, bias=0.0,
                                 scale=1.0)
            sl = slice(m * 128, (m + 1) * 128)
            nc.sync.dma_start(rowsc[b, sl], row1[:])
            nc.scalar.dma_start(
                RIs[b][:, sl],
                rowsc[b, sl].unsqueeze(0).to_broadcast((128, 128)))

        def emit_evict(b, m, tail=False):
            # sc = raw * RI  (all-bf16 SBUF), ob = 1 + sc*rinv_i -> f16
            n0 = m * 128
            W = T - n0
            sc = sc_pool.tile([128, W], BF16, tag="sc", name=f"sc_{b}_{m}",
                              padded_shape=[128, T])
            nc.vector.tensor_tensor(
                sc[:], raws[b][:, ROW_OFF[m]:ROW_OFF[m] + W],
                RIs[b][:, n0:], ALU.mult)
            ob = ob_pool.tile([128, W], F16, tag="ob", name=f"ob_{b}_{m}",
                              padded_shape=[128, T])
            if tail:
                dve_ob = 3 <= m <= 5
                pool_ob = m >= 6
            else:
                dve_ob = False
                pool_ob = m >= 2
            if dve_ob:
                nc.vector.tensor_scalar(ob[:], sc[:],
                                        nrivs[b][:, m:m + 1], 1.0,
                                        ALU.mult, ALU.add)
            elif pool_ob:
                nc.gpsimd.tensor_scalar(ob[:], sc[:],
                                        nrivs[b][:, m:m + 1], 1.0,
                                        ALU.mult, ALU.add)
            else:
                nc.scalar.activation(ob[:], sc[:], AF.Copy, bias=1.0,
                                     scale=nrivs[b][:, m:m + 1])
            eng = nc.sync if m % 2 == 0 else nc.scalar
            eng.dma_start(out[b, n0:n0 + 128, n0:], ob[:])

        # ---- pipeline ----
        emit_load(0)
        if BPC > 1:
            emit_load(1)
        last = BPC - 1
        for b in range(BPC):
            alloc_batch(b)
            if b != last:
                for m in range(TT):
                    if b + 2 < BPC and m == 5:
                        emit_load(b + 2)
                    pe_group(b, m)
                    if m == TT - 1:
                        emit_norm_head(b)
                    if b > 0:
                        if m == 1:
                            emit_transpose(b - 1)
                            emit_norm_tail(b - 1)
                        for mm_ in ([0] if m == 3 else [1, 2] if m == 4 else
                                    [3, 4] if m == 5 else [5, 6] if m == 6
                                    else [7] if m == 7 else []):
                            emit_evict(b - 1, mm_)
            else:
                # reverse order + per-block chains: block m's RI is
                # broadcast ~1 group after its gram, so this batch's
                # evictions (which need RI blocks m..7) overlap its own
                # compute instead of trailing the kernel.
                prev_tr = None
                prev_ev = None
                for i, m in enumerate(range(TT - 1, -1, -1)):
                    pe_group(b, m)
                    emit_col_chain(b, m)
                    if prev_tr is not None:
                        emit_col_transpose(b, prev_tr)
                        if prev_ev is not None:
                            emit_evict(b, prev_ev, tail=True)
                        prev_ev = prev_tr
                    prev_tr = m
                    if i == 1:
                        emit_transpose(b - 1)
                        emit_norm_tail(b - 1)
                    for mm_ in ([0] if i == 3 else [1, 2] if i == 4 else
                                [3, 4] if i == 5 else [5, 6] if i == 6
                                else [7] if i == 7 else []):
                        emit_evict(b - 1, mm_)
                emit_col_transpose(last, prev_tr)
                emit_evict(last, prev_ev, tail=True)
                emit_evict(last, prev_tr, tail=True)

    n = dedup_ldweights(nc)
    assert n > 0, "expected to remove redundant LDWEIGHTS"
    nc.compile()
    return nc


_MIRROR_MASK = None


def host_post(upper_f16):
    """Mirror the upper triangle onto the (unwritten) lower half, f32."""
    global _MIRROR_MASK
    if _MIRROR_MASK is None:
        idx = np.arange(T)
        _MIRROR_MASK = (idx[None, :] >= idx[:, None])[None]  # j >= i
    u = upper_f16.astype(np.float32)
    return np.where(_MIRROR_MASK, u, u.transpose(0, 2, 1))


def host_prep(x):
    x = np.asarray(x)
    x8 = x.astype(ml_dtypes.float8_e4m3)               # [B, T, C]
    xT8 = np.ascontiguousarray(x8.transpose(0, 2, 1))  # [B, C, T]
    return xT8


def run(x, trace=False):
    nc = build()
    xT8 = host_prep(x)
    in_maps = [{"xT8": xT8[i * BPC:(i + 1) * BPC]} for i in range(N_CORES)]
    last_err = None
    for _attempt in range(3):
        try:
            res = run_bass_kernel_spmd(nc, in_maps, list(range(N_CORES)),
                                       trace=trace)
            break
        except Exception as e:  # transient device wedge: retry
            last_err = e
            time.sleep(2.0)
    else:
        raise last_err
    out = np.concatenate([host_post(res.results[i]["out"])
                          for i in range(N_CORES)], axis=0)
    return out, res


def kernel(x):
    out, _ = run(x, trace=False)
    return out


# revision 17
# speedup vs baseline: 1.2565x; 1.2565x over previous
"""Trainium2 Bass kernel: per-batch cosine-distance matrix.

out[b] = 1 - metric[b] @ metric[b].T   where metric = x / ||x||_2 (last dim)
x: [32, 1024, 768] f32  ->  out: [32, 1024, 1024] f32

Sharding: data-parallel over batch. 8 cores x 4 batches each; no
cross-core communication.

Design (raw-fp8 gram, SBUF-staged scale-at-eviction; v8):
  Host prep: cast x to fp8 e4m3, transpose each batch to xT8 [C, T]
  (layout/dtype prep only - all math runs on device). Per batch,
  row blocks m = 0..7 (upper triangle incl. diagonal, cols m*128..T):
    1. DMA xT8 -> SBUF x83 [128, (k, t)] fp8, 6 chunked DMAs over the
       SP+ACT hwdge queues.
    2. PE: gram row m on RAW x8 (no met8 prescale): psum chunks of
       <=512 f32. fp8 DoubleRow K=256/matmul, j-outer so the chunks of
       a (m, j) pair share one LDWEIGHTS: the tile legalizer emits one
       LDWEIGHTS per matmul, so dedup_ldweights() strips the redundant
       loads post-legalize (migrating waits/deps); every PE instruction
       is chained (chain_iter_dep) so the final PE order is exactly
       emission order, making the weight reuse sound.
    3. Each psum chunk is immediately staged to SBUF bf16 (DVE
       tensor_copy / ACT copy split) into raw[b] [128, 4608] - psum
       slots recycle within ~1us, so the PE never waits on the
       normalization chain. 1-bank chunk slots, bufs=6.
    4. Norms: DVE stt-vs-identity on the psum diagonal block
       -> rv[:, m] = ss. Once per batch: DVE reciprocal (rr = 1/ss),
       then PE transpose of rr [128,8]->[8,128] - its only dependency
       is the DVE reciprocal, so the chained PE never waits on the busy
       ACT queue; the sqrt rides the row8 copy AFTER the transpose:
       row8 = Sqrt(rr^T) = +rinv bf16 -> DRAM -> partition-broadcast
       -> RI [128, T] bf16. nrivs = -rinv f32 is the per-partition
       eviction scale (two small ACT ops, off the critical path).
    5. Evictions of batch b run during batch b+1, emitted 3+ iterations
       in (avoids DVE FIFO head-of-line on RI): DVE tensor_tensor
       sc = raw * RI (all-bf16 SBUF), then ob = sc * nrivs[:, m] + 1.0
       -> f16 on ACT (rows 0-1, activation with AP scale) or Pool
       (rows 2-7, tensor_scalar) to balance engines; DMA out on
       alternating queues. The final batch's evictions (the tail) are
       split ACT/DVE/Pool three ways.
  Host post: upcast f16 -> f32 and mirror the (symmetric) lower half.

Measured on 8 axon trn2 cores: 76.8-77.6 us HW exec (met8 baseline:
87.3-89.8 us), rel err 1.38e-3 (budget 2e-2). Engine busy at 77 us:
DVE 60% / PE 47% (+20% LDWEIGHTS) / ACT 35% / Pool 20%.
"""

import sys
import time
from contextlib import ExitStack

_TRN_REPO = "/opt/trn_rl_repo"
if _TRN_REPO not in sys.path:
    sys.path.insert(0, _TRN_REPO)

import numpy as np
import ml_dtypes

import concourse.bacc as bacc
import concourse.mybir as mybir
import concourse.tile as tile
from concourse.bass_utils import run_bass_kernel_spmd
from concourse.masks import make_identity

B, T, C = 32, 1024, 768
N_CORES = 8
BPC = B // N_CORES   # batches per core
KC = C // 128        # 6 k-chunks
KP = KC // 2         # 3 k-pairs (DoubleRow)
TT = T // 128        # 8 row blocks
F32 = mybir.dt.float32
F16 = mybir.dt.float16
BF16 = mybir.dt.bfloat16
F8 = mybir.dt.float8e4
AF = mybir.ActivationFunctionType
ALU = mybir.AluOpType
DR = mybir.MatmulPerfMode.DoubleRow

ROW_OFF = [0]
for _m in range(TT):
    ROW_OFF.append(ROW_OFF[-1] + (T - _m * 128))  # prefix offsets into raw


def dedup_ldweights(nc):
    """Remove InstLdweights whose weights AP equals the currently-loaded
    stationary (set by the previous LDW and not clobbered since). Runs
    after tile legalization (final instruction order) and before
    nc.compile() (semaphore generation), so migrating the removed LDW's
    sync_info and dependency edges onto the following matmul is safe.
    """
    removed = 0
    for f in nc.m.functions:
        for bb in f.blocks:
            keep = []
            last_key = None
            donors = []
            for inst in bb.instructions:
                if getattr(inst, "engine", None) != mybir.EngineType.PE:
                    keep.append(inst)
                    continue
                if isinstance(inst, mybir.InstLdweights):
                    try:
                        key = (str(inst.ins[0]), str(inst.perf_mode))
                    except Exception:
                        key = None
                    if key is not None and key == last_key:
                        donors.append(inst)
                        removed += 1
                        continue
                    last_key = key
                    keep.append(inst)
                else:
                    if isinstance(inst, mybir.InstMatmult) and inst.is_transpose:
                        last_key = None
                    elif not isinstance(inst, mybir.InstMatmult):
                        last_key = None
                    for d in donors:
                        inst.merge_dependencies_from(d)
                        dsi = d.sync_info
                        if dsi is not None and (dsi.on_wait or dsi.on_update):
                            si = inst.sync_info
                            ow = list(si.on_wait) if si else []
                            ou = list(si.on_update) if si else []
                            inst.sync_info = mybir.SyncInfo(
                                on_wait=ow + list(dsi.on_wait),
                                on_update=ou + list(dsi.on_update))
                    donors = []
                    keep.append(inst)
            assert not donors, "trailing removed LDW with no successor"
            bb.instructions[:] = keep
    return removed


def build():
    nc = bacc.Bacc("TRN2", target_bir_lowering=False, debug=False,
                   num_devices=N_CORES)
    xT8 = nc.dram_tensor("xT8", [BPC, C, T], F8, kind="ExternalInput").ap()
    out = nc.dram_tensor("out", [BPC, T, T], F16, kind="ExternalOutput").ap()
    rowsc = nc.dram_tensor("rowsc", [BPC, T], BF16, kind="Internal").ap()

    with tile.TileContext(nc) as tc, ExitStack() as ctx:
        x_pool = ctx.enter_context(tc.tile_pool(name="x", bufs=3))
        raw_pool = ctx.enter_context(tc.tile_pool(name="raw", bufs=2))
        ri_pool = ctx.enter_context(tc.tile_pool(name="ri", bufs=2))
        s_pool = ctx.enter_context(tc.tile_pool(name="s", bufs=2))
        sc_pool = ctx.enter_context(tc.tile_pool(name="sc", bufs=3))
        ob_pool = ctx.enter_context(tc.tile_pool(name="ob", bufs=4))
        c_pool = ctx.enter_context(tc.tile_pool(name="c", bufs=1))
        ps_pool = ctx.enter_context(
            tc.tile_pool(name="ps", bufs=6, space="PSUM"))  # 1 bank/slot
        psT_pool = ctx.enter_context(
            tc.tile_pool(name="psT", bufs=2, space="PSUM"))

        identf = c_pool.tile([128, 128], F32)
        make_identity(nc, identf[:])
        dummyf = c_pool.tile([128, 128], F32, tag="dummyf")

        # warm the ACT Sqrt table while the first DMA flies
        warm = c_pool.tile([128, 1], F32, tag="warm")
        nc.vector.memset(warm[:], 1.0)
        warm2 = c_pool.tile([128, 1], F32, tag="warm2")
        nc.scalar.sqrt(warm2[:], warm[:])

        x83s, raws, RIs, rvs, rrs, rivs, nrivs, rvTs = ({}, {}, {}, {}, {}, {}, {}, {})

        def chain(inst):
            tc.chain_iter_dep("pe_order", inst.ins)

        def emit_load(b):
            x8 = x_pool.tile([128, KC * T], F8, tag="x8", name=f"x8_{b}")
            x83 = x8[:].rearrange("p (k t) -> p k t", k=KC)
            src = xT8[b].rearrange("(k p) t -> p k t", p=128)
            for k in range(KC):
                eng = nc.sync if k % 2 == 0 else nc.scalar
                eng.dma_start(x83[:, k, :], src[:, k, :])
            x83s[b] = x83

        def alloc_batch(b):
            raws[b] = raw_pool.tile([128, ROW_OFF[TT]], BF16, tag="raw",
                                    name=f"raw_{b}")
            RIs[b] = ri_pool.tile([128, T], BF16, tag="RI", name=f"RI_{b}")
            rvs[b] = s_pool.tile([128, TT], F32, tag="rv", name=f"rv_{b}")
            rrs[b] = s_pool.tile([128, TT], F32, tag="rr", name=f"rr_{b}")
            rivs[b] = s_pool.tile([128, TT], F32, tag="riv", name=f"riv_{b}")
            nrivs[b] = s_pool.tile([128, TT], F32, tag="nriv",
                                   name=f"nriv_{b}")

        def chunks_of(m):
            W = T - m * 128
            ch = [(0, min(512, W))]
            if W > 512:
                ch.append((512, W - 512))
            return ch

        def emit_diag_pass(b):
            # last batch only: compute all 8 diagonal blocks up front so
            # the norm chain (and hence this batch's evictions) overlaps
            # this batch's own gram pass instead of trailing the kernel.
            x83 = x83s[b]
            dts = [ps_pool.tile([128, 512], F32, tag="pg",
                                name=f"dg_{b}_{h}") for h in range(2)]
            for m in range(TT):
                dst = dts[m // 4][:, (m % 4) * 128:(m % 4) * 128 + 128]
                msl = slice(m * 128, (m + 1) * 128)
                for j in range(KP):
                    mm = nc.tensor.matmul(
                        dst, x83[:, 2 * j:2 * j + 2, msl],
                        x83[:, 2 * j:2 * j + 2, msl],
                        start=(j == 0), stop=(j == KP - 1),
                        perf_mode=DR, skip_group_check=True)
                    chain(mm)
                nc.vector.scalar_tensor_tensor(
                    dummyf[:], dst, 1.0, identf[:], ALU.mult,
                    ALU.mult, accum_out=rvs[b][:, m:m + 1])

        def pe_group(b, m, extract=True):
            # gram row m on raw x8, j-outer; chunk tiles are 1 psum bank;
            # each chunk staged to SBUF bf16 right after its stop matmul.
            x83 = x83s[b]
            n0 = m * 128
            tiles = []
            for ci, (off, w) in enumerate(chunks_of(m)):
                pgt = ps_pool.tile([128, w], F32, tag="pg",
                                   name=f"pg_{b}_{m}_{ci}",
                                   padded_shape=[128, 512])
                tiles.append((pgt, off, w))
            pgs0 = tiles[0][0]
            msl = slice(n0, n0 + 128)
            for j in range(KP):
                for pgt, off, w in tiles:
                    mm = nc.tensor.matmul(
                        pgt[:],
                        x83[:, 2 * j:2 * j + 2, msl],
                        x83[:, 2 * j:2 * j + 2, n0 + off:n0 + off + w],
                        start=(j == 0), stop=(j == KP - 1),
                        perf_mode=DR, skip_group_check=True)
                    chain(mm)
            if extract:
                # ss from the psum diagonal block, before staging
                nc.vector.scalar_tensor_tensor(
                    dummyf[:], pgs0[:, 0:128], 1.0, identf[:], ALU.mult,
                    ALU.mult, accum_out=rvs[b][:, m:m + 1])
            # stage to SBUF: narrow rows via ACT, wide chunks via DVE
            for pgt, off, w in tiles:
                dst = raws[b][:, ROW_OFF[m] + off:ROW_OFF[m] + off + w]
                if m >= 4:
                    nc.scalar.activation(dst, pgt[:], AF.Copy, bias=0.0,
                                         scale=1.0)
                else:
                    nc.vector.tensor_copy(dst, pgt[:])

        def emit_norm_head(b):
            # once per batch, right after the last extract. Only the DVE
            # reciprocal sits ahead of the PE transpose; sqrt happens
            # after the transpose so the chained PE never waits on ACT.
            nc.vector.reciprocal(rrs[b][:], rvs[b][:])

        def emit_norm_tail(b):
            # PE transpose (of rr = 1/ss) is emitted by the caller; here:
            # row8 = sqrt(rr)^T = +rinv (bf16), DRAM round trip, partition
            # broadcast -> RI (positive). nrivs = -rinv is the per-partition
            # eviction scale; neither is on the PE critical path.
            row8 = s_pool.tile([TT, 128], BF16, tag="row8",
                               name=f"row8_{b}")
            nc.scalar.activation(row8[:], rvTs[b][:], AF.Sqrt, bias=0.0,
                                 scale=1.0)
            nc.sync.dma_start(rowsc[b], row8[:])
            nc.scalar.dma_start(
                RIs[b][:], rowsc[b].unsqueeze(0).to_broadcast((128, T)))
            nc.scalar.activation(rivs[b][:], rrs[b][:], AF.Sqrt, bias=0.0,
                                 scale=1.0)
            nc.scalar.activation(nrivs[b][:], rivs[b][:], AF.Copy, bias=0.0,
                                 scale=-1.0)

        def emit_transpose(b):
            rvT = psT_pool.tile([TT, 128], F32, tag="rvT", name=f"rvT_{b}")
            mmT = nc.tensor.transpose(rvT[:], rrs[b][:], identf[:])
            chain(mmT)
            rvTs[b] = rvT

        def emit_evict(b, m, tail=False):
            # sc = raw * RI  (all-bf16 SBUF), ob = 1 + sc*rinv_i -> f16
            n0 = m * 128
            W = T - n0
            sc = sc_pool.tile([128, W], BF16, tag="sc", name=f"sc_{b}_{m}",
                              padded_shape=[128, T])
            nc.vector.tensor_tensor(
                sc[:], raws[b][:, ROW_OFF[m]:ROW_OFF[m] + W],
                RIs[b][:, n0:], ALU.mult)
            ob = ob_pool.tile([128, W], F16, tag="ob", name=f"ob_{b}_{m}",
                              padded_shape=[128, T])
            if tail:
                dve_ob = 3 <= m <= 5
                pool_ob = m >= 6
            else:
                dve_ob = False
                pool_ob = m >= 2
            if dve_ob:
                nc.vector.tensor_scalar(ob[:], sc[:],
                                        nrivs[b][:, m:m + 1], 1.0,
                                        ALU.mult, ALU.add)
            elif pool_ob:
                nc.gpsimd.tensor_scalar(ob[:], sc[:],
                                        nrivs[b][:, m:m + 1], 1.0,
                                        ALU.mult, ALU.add)
            else:
                nc.scalar.activation(ob[:], sc[:], AF.Copy, bias=1.0,
                                     scale=nrivs[b][:, m:m + 1])
            eng = nc.sync if m % 2 == 0 else nc.scalar
            eng.dma_start(out[b, n0:n0 + 128, n0:], ob[:])

        # ---- pipeline ----
        emit_load(0)
        if BPC > 1:
            emit_load(1)
        last = BPC - 1
        for b in range(BPC):
            alloc_batch(b)
            for m in range(TT):
                if b + 2 < BPC and m == 5:
                    emit_load(b + 2)
                pe_group(b, m)
                if m == TT - 1:
                    emit_norm_head(b)
                if b > 0:
                    if m == 1:
                        emit_transpose(b - 1)
                        emit_norm_tail(b - 1)
                    for mm_ in ([0] if m == 3 else [1, 2] if m == 4 else
                                [3, 4] if m == 5 else [5, 6] if m == 6 else
                                [7] if m == 7 else []):
                        emit_evict(b - 1, mm_)
        # epilogue: last batch's chain + evictions (3-way ob split)
        emit_transpose(last)
        emit_norm_tail(last)
        for m in range(TT):
            emit_evict(last, m, tail=True)

    n = dedup_ldweights(nc)
    assert n > 0, "expected to remove redundant LDWEIGHTS"
    nc.compile()
    return nc


_MIRROR_MASK = None


def host_post(upper_f16):
    """Mirror the upper triangle onto the (unwritten) lower half, f32."""
    global _MIRROR_MASK
    if _MIRROR_MASK is None:
        idx = np.arange(T)
        _MIRROR_MASK = (idx[None, :] >= idx[:, None])[None]  # j >= i
    u = upper_f16.astype(np.float32)
    return np.where(_MIRROR_MASK, u, u.transpose(0, 2, 1))


def host_prep(x):
    x = np.asarray(x)
    x8 = x.astype(ml_dtypes.float8_e4m3)               # [B, T, C]
    xT8 = np.ascontiguousarray(x8.transpose(0, 2, 1))  # [B, C, T]
    return xT8


def run(x, trace=False):
    nc = build()
    xT8 = host_prep(x)
    in_maps = [{"xT8": xT8[i * BPC:(i + 1) * BPC]} for i in range(N_CORES)]
    last_err = None
    for _attempt in range(3):
        try:
            res = run_bass_kernel_spmd(nc, in_maps, list(range(N_CORES)),
                                       trace=trace)
            break
        except Exception as e:  # transient device wedge: retry
            last_err = e
            time.sleep(2.0)
    else:
        raise last_err
    out = np.concatenate([host_post(res.results[i]["out"])
                          for i in range(N_CORES)], axis=0)
    return out, res


def kernel(x):
    out, _ = run(x, trace=False)
    return out


# revision 18
# speedup vs baseline: 1.2619x; 1.0043x over previous
"""Trainium2 Bass kernel: per-batch cosine-distance matrix.

out[b] = 1 - metric[b] @ metric[b].T   where metric = x / ||x||_2 (last dim)
x: [32, 1024, 768] f32  ->  out: [32, 1024, 1024] f32

Sharding: data-parallel over batch. 8 cores x 4 batches each; no
cross-core communication.

Design (raw-fp8 gram, SBUF-staged scale-at-eviction; v8):
  Host prep: cast x to fp8 e4m3, transpose each batch to xT8 [C, T]
  (layout/dtype prep only - all math runs on device). Per batch,
  row blocks m = 0..7 (upper triangle incl. diagonal, cols m*128..T):
    1. DMA xT8 -> SBUF x83 [128, (k, t)] fp8, 6 chunked DMAs over the
       SP+ACT hwdge queues.
    2. PE: gram row m on RAW x8 (no met8 prescale): psum chunks of
       <=512 f32. fp8 DoubleRow K=256/matmul, j-outer so the chunks of
       a (m, j) pair share one LDWEIGHTS: the tile legalizer emits one
       LDWEIGHTS per matmul, so dedup_ldweights() strips the redundant
       loads post-legalize (migrating waits/deps); every PE instruction
       is chained (chain_iter_dep) so the final PE order is exactly
       emission order, making the weight reuse sound.
    3. Each psum chunk is immediately staged to SBUF bf16 (DVE
       tensor_copy / ACT copy split) into raw[b] [128, 4608] - psum
       slots recycle within ~1us, so the PE never waits on the
       normalization chain. 1-bank chunk slots, bufs=6.
    4. Norms: DVE stt-vs-identity on the psum diagonal block
       -> rv[:, m] = ss. Once per batch: DVE reciprocal (rr = 1/ss),
       then PE transpose of rr [128,8]->[8,128] - its only dependency
       is the DVE reciprocal, so the chained PE never waits on the busy
       ACT queue; the sqrt rides the row8 copy AFTER the transpose:
       row8 = Sqrt(rr^T) = +rinv bf16 -> DRAM -> partition-broadcast
       -> RI [128, T] bf16. nrivs = -rinv f32 is the per-partition
       eviction scale (two small ACT ops, off the critical path).
    5. Evictions of batch b run during batch b+1, emitted 3+ iterations
       in (avoids DVE FIFO head-of-line on RI): DVE tensor_tensor
       sc = raw * RI (all-bf16 SBUF), then ob = sc * nrivs[:, m] + 1.0
       -> f16 on ACT (rows 0-1, activation with AP scale) or Pool
       (rows 2-7, tensor_scalar) to balance engines; DMA out on
       alternating queues. The final batch's evictions (the tail) are
       split ACT/DVE/Pool three ways.
  Host post: upcast f16 -> f32 and mirror the (symmetric) lower half.

Measured on 8 axon trn2 cores: 76.8-77.6 us HW exec (met8 baseline:
87.3-89.8 us), rel err 1.38e-3 (budget 2e-2). Engine busy at 77 us:
DVE 60% / PE 47% (+20% LDWEIGHTS) / ACT 35% / Pool 20%.
"""

import sys
import time
from contextlib import ExitStack

_TRN_REPO = "/opt/trn_rl_repo"
if _TRN_REPO not in sys.path:
    sys.path.insert(0, _TRN_REPO)

import numpy as np
import ml_dtypes

import concourse.bacc as bacc
import concourse.mybir as mybir
import concourse.tile as tile
from concourse.bass_utils import run_bass_kernel_spmd
from concourse.masks import make_identity

B, T, C = 32, 1024, 768
N_CORES = 8
BPC = B // N_CORES   # batches per core
KC = C // 128        # 6 k-chunks
KP = KC // 2         # 3 k-pairs (DoubleRow)
TT = T // 128        # 8 row blocks
F32 = mybir.dt.float32
F16 = mybir.dt.float16
BF16 = mybir.dt.bfloat16
F8 = mybir.dt.float8e4
AF = mybir.ActivationFunctionType
ALU = mybir.AluOpType
DR = mybir.MatmulPerfMode.DoubleRow

ROW_OFF = [0]
for _m in range(TT):
    ROW_OFF.append(ROW_OFF[-1] + (T - _m * 128))  # prefix offsets into raw


def dedup_ldweights(nc):
    """Remove InstLdweights whose weights AP equals the currently-loaded
    stationary (set by the previous LDW and not clobbered since). Runs
    after tile legalization (final instruction order) and before
    nc.compile() (semaphore generation), so migrating the removed LDW's
    sync_info and dependency edges onto the following matmul is safe.
    """
    removed = 0
    for f in nc.m.functions:
        for bb in f.blocks:
            keep = []
            last_key = None
            donors = []
            for inst in bb.instructions:
                if getattr(inst, "engine", None) != mybir.EngineType.PE:
                    keep.append(inst)
                    continue
                if isinstance(inst, mybir.InstLdweights):
                    try:
                        key = (str(inst.ins[0]), str(inst.perf_mode))
                    except Exception:
                        key = None
                    if key is not None and key == last_key:
                        donors.append(inst)
                        removed += 1
                        continue
                    last_key = key
                    keep.append(inst)
                else:
                    if isinstance(inst, mybir.InstMatmult) and inst.is_transpose:
                        last_key = None
                    elif not isinstance(inst, mybir.InstMatmult):
                        last_key = None
                    for d in donors:
                        inst.merge_dependencies_from(d)
                        dsi = d.sync_info
                        if dsi is not None and (dsi.on_wait or dsi.on_update):
                            si = inst.sync_info
                            ow = list(si.on_wait) if si else []
                            ou = list(si.on_update) if si else []
                            inst.sync_info = mybir.SyncInfo(
                                on_wait=ow + list(dsi.on_wait),
                                on_update=ou + list(dsi.on_update))
                    donors = []
                    keep.append(inst)
            assert not donors, "trailing removed LDW with no successor"
            bb.instructions[:] = keep
    return removed


def build():
    nc = bacc.Bacc("TRN2", target_bir_lowering=False, debug=False,
                   num_devices=N_CORES)
    xT8 = nc.dram_tensor("xT8", [BPC, C, T], F8, kind="ExternalInput").ap()
    out = nc.dram_tensor("out", [BPC, T, T], F16, kind="ExternalOutput").ap()
    rowsc = nc.dram_tensor("rowsc", [BPC, T], BF16, kind="Internal").ap()

    with tile.TileContext(nc) as tc, ExitStack() as ctx:
        x_pool = ctx.enter_context(tc.tile_pool(name="x", bufs=3))
        raw_pool = ctx.enter_context(tc.tile_pool(name="raw", bufs=2))
        ri_pool = ctx.enter_context(tc.tile_pool(name="ri", bufs=2))
        s_pool = ctx.enter_context(tc.tile_pool(name="s", bufs=2))
        sc_pool = ctx.enter_context(tc.tile_pool(name="sc", bufs=3))
        ob_pool = ctx.enter_context(tc.tile_pool(name="ob", bufs=4))
        c_pool = ctx.enter_context(tc.tile_pool(name="c", bufs=1))
        ps_pool = ctx.enter_context(
            tc.tile_pool(name="ps", bufs=6, space="PSUM"))  # 1 bank/slot
        psT_pool = ctx.enter_context(
            tc.tile_pool(name="psT", bufs=2, space="PSUM"))

        identf = c_pool.tile([128, 128], F32)
        make_identity(nc, identf[:])
        dummyf = c_pool.tile([128, 128], F32, tag="dummyf")

        # warm the ACT Sqrt table while the first DMA flies
        warm = c_pool.tile([128, 1], F32, tag="warm")
        nc.vector.memset(warm[:], 1.0)
        warm2 = c_pool.tile([128, 1], F32, tag="warm2")
        nc.scalar.sqrt(warm2[:], warm[:])

        x83s, raws, RIs, rvs, rrs, rivs, nrivs, rvTs = ({}, {}, {}, {}, {}, {}, {}, {})

        def chain(inst):
            tc.chain_iter_dep("pe_order", inst.ins)

        def emit_load(b):
            x8 = x_pool.tile([128, KC * T], F8, tag="x8", name=f"x8_{b}")
            x83 = x8[:].rearrange("p (k t) -> p k t", k=KC)
            src = xT8[b].rearrange("(k p) t -> p k t", p=128)
            for k in range(KC):
                eng = nc.sync if k % 2 == 0 else nc.scalar
                eng.dma_start(x83[:, k, :], src[:, k, :])
            x83s[b] = x83

        def alloc_batch(b):
            raws[b] = raw_pool.tile([128, ROW_OFF[TT]], BF16, tag="raw",
                                    name=f"raw_{b}")
            RIs[b] = ri_pool.tile([128, T], BF16, tag="RI", name=f"RI_{b}")
            rvs[b] = s_pool.tile([128, TT], F32, tag="rv", name=f"rv_{b}")
            rrs[b] = s_pool.tile([128, TT], F32, tag="rr", name=f"rr_{b}")
            rivs[b] = s_pool.tile([128, TT], F32, tag="riv", name=f"riv_{b}")
            nrivs[b] = s_pool.tile([128, TT], F32, tag="nriv",
                                   name=f"nriv_{b}")

        def chunks_of(m):
            W = T - m * 128
            ch = [(0, min(512, W))]
            if W > 512:
                ch.append((512, W - 512))
            return ch

        def emit_diag_pass(b):
            # last batch only: compute all 8 diagonal blocks up front so
            # the norm chain (and hence this batch's evictions) overlaps
            # this batch's own gram pass instead of trailing the kernel.
            x83 = x83s[b]
            dts = [ps_pool.tile([128, 512], F32, tag="pg",
                                name=f"dg_{b}_{h}") for h in range(2)]
            for m in range(TT):
                dst = dts[m // 4][:, (m % 4) * 128:(m % 4) * 128 + 128]
                msl = slice(m * 128, (m + 1) * 128)
                for j in range(KP):
                    mm = nc.tensor.matmul(
                        dst, x83[:, 2 * j:2 * j + 2, msl],
                        x83[:, 2 * j:2 * j + 2, msl],
                        start=(j == 0), stop=(j == KP - 1),
                        perf_mode=DR, skip_group_check=True)
                    chain(mm)
                nc.vector.scalar_tensor_tensor(
                    dummyf[:], dst, 1.0, identf[:], ALU.mult,
                    ALU.mult, accum_out=rvs[b][:, m:m + 1])

        def pe_group(b, m, extract=True):
            # gram row m on raw x8, j-outer; chunk tiles are 1 psum bank;
            # each chunk staged to SBUF bf16 right after its stop matmul.
            x83 = x83s[b]
            n0 = m * 128
            tiles = []
            for ci, (off, w) in enumerate(chunks_of(m)):
                pgt = ps_pool.tile([128, w], F32, tag="pg",
                                   name=f"pg_{b}_{m}_{ci}",
                                   padded_shape=[128, 512])
                tiles.append((pgt, off, w))
            pgs0 = tiles[0][0]
            msl = slice(n0, n0 + 128)
            for j in range(KP):
                for pgt, off, w in tiles:
                    mm = nc.tensor.matmul(
                        pgt[:],
                        x83[:, 2 * j:2 * j + 2, msl],
                        x83[:, 2 * j:2 * j + 2, n0 + off:n0 + off + w],
                        start=(j == 0), stop=(j == KP - 1),
                        perf_mode=DR, skip_group_check=True)
                    chain(mm)
            if extract:
                # ss from the psum diagonal block, before staging
                nc.vector.scalar_tensor_tensor(
                    dummyf[:], pgs0[:, 0:128], 1.0, identf[:], ALU.mult,
                    ALU.mult, accum_out=rvs[b][:, m:m + 1])
            # stage to SBUF: narrow rows via ACT, wide chunks via DVE
            for pgt, off, w in tiles:
                dst = raws[b][:, ROW_OFF[m] + off:ROW_OFF[m] + off + w]
                if m >= 4:
                    nc.scalar.activation(dst, pgt[:], AF.Copy, bias=0.0,
                                         scale=1.0)
                else:
                    nc.vector.tensor_copy(dst, pgt[:])

        def emit_norm_head(b):
            # once per batch, right after the last extract. Only the DVE
            # reciprocal sits ahead of the PE transpose; sqrt happens
            # after the transpose so the chained PE never waits on ACT.
            nc.vector.reciprocal(rrs[b][:], rvs[b][:])

        def emit_norm_tail(b):
            # PE transpose (of rr = 1/ss) is emitted by the caller; here:
            # row8 = sqrt(rr)^T = +rinv (bf16), DRAM round trip, partition
            # broadcast -> RI (positive). nrivs = -rinv is the per-partition
            # eviction scale; neither is on the PE critical path.
            row8 = s_pool.tile([TT, 128], BF16, tag="row8",
                               name=f"row8_{b}")
            nc.scalar.activation(row8[:], rvTs[b][:], AF.Sqrt, bias=0.0,
                                 scale=1.0)
            nc.sync.dma_start(rowsc[b], row8[:])
            nc.scalar.dma_start(
                RIs[b][:], rowsc[b].unsqueeze(0).to_broadcast((128, T)))
            nc.scalar.activation(rivs[b][:], rrs[b][:], AF.Sqrt, bias=0.0,
                                 scale=1.0)
            nc.scalar.activation(nrivs[b][:], rivs[b][:], AF.Copy, bias=0.0,
                                 scale=-1.0)

        def emit_transpose(b):
            rvT = psT_pool.tile([TT, 128], F32, tag="rvT", name=f"rvT_{b}")
            mmT = nc.tensor.transpose(rvT[:], rrs[b][:], identf[:])
            chain(mmT)
            rvTs[b] = rvT

        def emit_evict(b, m, tail=False):
            # sc = raw * RI  (all-bf16 SBUF), ob = 1 + sc*rinv_i -> f16
            n0 = m * 128
            W = T - n0
            sc = sc_pool.tile([128, W], BF16, tag="sc", name=f"sc_{b}_{m}",
                              padded_shape=[128, T])
            nc.vector.tensor_tensor(
                sc[:], raws[b][:, ROW_OFF[m]:ROW_OFF[m] + W],
                RIs[b][:, n0:], ALU.mult)
            ob = ob_pool.tile([128, W], F16, tag="ob", name=f"ob_{b}_{m}",
                              padded_shape=[128, T])
            if tail:
                dve_ob = False
                pool_ob = m >= 3
            else:
                dve_ob = False
                pool_ob = m >= 2
            if dve_ob:
                nc.vector.tensor_scalar(ob[:], sc[:],
                                        nrivs[b][:, m:m + 1], 1.0,
                                        ALU.mult, ALU.add)
            elif pool_ob:
                nc.gpsimd.tensor_scalar(ob[:], sc[:],
                                        nrivs[b][:, m:m + 1], 1.0,
                                        ALU.mult, ALU.add)
            else:
                nc.scalar.activation(ob[:], sc[:], AF.Copy, bias=1.0,
                                     scale=nrivs[b][:, m:m + 1])
            eng = nc.sync if m % 2 == 0 else nc.scalar
            eng.dma_start(out[b, n0:n0 + 128, n0:], ob[:])

        # ---- pipeline ----
        emit_load(0)
        if BPC > 1:
            emit_load(1)
        last = BPC - 1
        for b in range(BPC):
            alloc_batch(b)
            for m in range(TT):
                if b + 2 < BPC and m == 5:
                    emit_load(b + 2)
                pe_group(b, m)
                if m == TT - 1:
                    emit_norm_head(b)
                if b > 0:
                    if m == 1:
                        emit_transpose(b - 1)
                        emit_norm_tail(b - 1)
                    for mm_ in ([0] if m == 3 else [1, 2] if m == 4 else
                                [3, 4] if m == 5 else [5, 6] if m == 6 else
                                [7] if m == 7 else []):
                        emit_evict(b - 1, mm_)
        # epilogue: last batch's chain + evictions (3-way ob split)
        emit_transpose(last)
        emit_norm_tail(last)
        for m in range(TT):
            emit_evict(last, m, tail=True)

    n = dedup_ldweights(nc)
    assert n > 0, "expected to remove redundant LDWEIGHTS"
    nc.compile()
    return nc


_MIRROR_MASK = None


def host_post(upper_f16):
    """Mirror the upper triangle onto the (unwritten) lower half, f32."""
    global _MIRROR_MASK
    if _MIRROR_MASK is None:
        idx = np.arange(T)
        _MIRROR_MASK = (idx[None, :] >= idx[:, None])[None]  # j >= i
    u = upper_f16.astype(np.float32)
    return np.where(_MIRROR_MASK, u, u.transpose(0, 2, 1))


def host_prep(x):
    x = np.asarray(x)
    x8 = x.astype(ml_dtypes.float8_e4m3)               # [B, T, C]
    xT8 = np.ascontiguousarray(x8.transpose(0, 2, 1))  # [B, C, T]
    return xT8


def run(x, trace=False):
    nc = build()
    xT8 = host_prep(x)
    in_maps = [{"xT8": xT8[i * BPC:(i + 1) * BPC]} for i in range(N_CORES)]
    last_err = None
    for _attempt in range(3):
        try:
            res = run_bass_kernel_spmd(nc, in_maps, list(range(N_CORES)),
                                       trace=trace)
            break
        except Exception as e:  # transient device wedge: retry
            last_err = e
            time.sleep(2.0)
    else:
        raise last_err
    out = np.concatenate([host_post(res.results[i]["out"])
                          for i in range(N_CORES)], axis=0)
    return out, res


def kernel(x):
    out, _ = run(x, trace=False)
    return out
